# revision 15
# baseline (speedup 1.0000x reference)
import sys

sys.path.insert(0, "/opt/trn_rl_repo")

import ml_dtypes
import numpy as np

from concourse import bass, tile, bacc
from concourse.bass_utils import run_bass_kernel_spmd

WORLD, M, N, K_LOC = 8, 8192, 2048, 256
M_PER = M // WORLD  # 1024
K_TOT = WORLD * K_LOC  # 2048
F32 = bass.mybir.dt.float32
BF16 = bass.mybir.dt.bfloat16
KC = K_TOT // 128  # 16 k-chunks of 128
MB = M_PER // 128  # 8 m-blocks of 128
NB = N // 512  # 4 psum-width chunks of 512

_cached_nc = None


def _build():
    # Output-sharded decomposition: core r computes the full contraction
    # (K_total = 2048) for its own M/8 row block, so no cross-core
    # reduce-scatter and no [M, N] partial ever hits HBM. Inputs arrive
    # pre-transposed K-major (and bf16) from the host.
    global _cached_nc
    if _cached_nc is not None:
        return _cached_nc
    nc = bacc.Bacc(None, target_bir_lowering=False, num_devices=WORLD)
    A = nc.dram_tensor("A", [K_TOT, M_PER], BF16, kind="ExternalInput")
    W = nc.dram_tensor("W", [K_TOT, N], BF16, kind="ExternalInput")
    # bf16 output staging halves the copy+store SBUF traffic that contends
    # with PE operand streaming; host upcasts back to f32 (err ~3e-3 << 2e-2)
    out = nc.dram_tensor("out", [M_PER, N], BF16, kind="ExternalOutput")

    NPAIR = MB // 2  # m-block pairs; each pair owns all 8 PSUM banks
    with tile.TileContext(nc) as tc:
        with (
            tc.tile_pool(name="wp", bufs=1) as wp,
            tc.tile_pool(name="ap", bufs=1) as apool,
            tc.tile_pool(name="ob", bufs=4) as ob,
            tc.tile_pool(name="ps", bufs=8, space=bass.MemorySpace.PSUM) as ps,
        ):
            # Both operands fully SBUF-resident: W^T 64KB/partition,
            # A^T 32KB/partition. Loads are issued in PE-consumption order
            # (pair 0 first, k-chunk-major) so matmuls start after the first
            # ~600KB instead of after the full 12.6MB.
            Wt = wp.tile([128, KC, N], BF16)
            At = apool.tile([128, KC, M_PER], BF16)

            # PE warm-up: junk matmuls bridge the load latency so the PE's
            # p-state ramp (2x slower first ~3us) is spent on throwaway work
            # that abuts the first real matmul.
            junk = wp.tile([128, 640], BF16)
            jacc = ps.tile([128, 512], F32, name="acc")
            nc.vector.memset(junk[:], 0)
            for _ in range(8):
                nc.tensor.matmul(
                    jacc[:], junk[:, 0:128], junk[:, 128:640], start=True, stop=True
                )

            q = [nc.sync, nc.scalar]
            # Critical prefix, k-chunk-major: pair 0's s-step consumes
            # (W chunk s, A cols 0:256 chunk s) — 576KB per step, just under
            # the PE's 1.77us/step appetite. Both DMAs of a step go on the
            # SAME engine (alternating per step) so the two HWDGE streams
            # stay interleaved in consumption order.
            for s in range(KC):
                eng = q[s % 2]
                if s == 0:
                    # First chunk fine-grained; the two first-needed pieces
                    # go via gpsimd/SWDGE, skipping the HWDGE queue.
                    nc.gpsimd.dma_start(At[:, 0, 0:256], A[0:128, 0:256])
                    nc.gpsimd.dma_start(Wt[:, 0, 0:512], W[0:128, 0:512])
                    for nb in range(1, NB):
                        eng.dma_start(
                            Wt[:, 0, nb * 512 : (nb + 1) * 512],
                            W[0:128, nb * 512 : (nb + 1) * 512],
                        )
                    continue
                eng.dma_start(Wt[:, s, :], W[s * 128 : (s + 1) * 128, :])
                # per-step A covers pair 0 (cols 0:256): 576KB/step, just
                # under the PE's 1.71us/step appetite
                eng.dma_start(At[:, s, 0:256], A[s * 128 : (s + 1) * 128, 0:256])
            # Pairs 1-3 arrive as one bulk DMA each, issued after each
            # engine's critical stream so they cannot jump ahead of late W
            # chunks: A1 on scalar (shorter stream, arrives just in time for
            # pair 1 at ~31us), A2/A3 on sync.
            nc.scalar.dma_start(
                At[:, :, 256:512],
                A[:, 256:512].rearrange("(s q) m -> q s m", q=128),
            )
            for p in range(2, NPAIR):
                nc.sync.dma_start(
                    At[:, :, p * 256 : (p + 1) * 256],
                    A[:, p * 256 : (p + 1) * 256].rearrange(
                        "(s q) m -> q s m", q=128
                    ),
                )

            for p in range(NPAIR):
                rows = [ob.tile([128, N], BF16, name="row") for _ in range(2)]
                accs = [ps.tile([128, 512], F32, name="acc") for _ in range(2 * NB)]
                if p < NPAIR - 1:
                    # s-major: all 8 chains advance per k-chunk, matching
                    # DMA arrival order (matters for pair 0)
                    for s in range(KC):
                        for h in range(2):
                            mb = 2 * p + h
                            for nb in range(NB):
                                nc.tensor.matmul(
                                    accs[h * NB + nb][:],
                                    At[:, s, mb * 128 : (mb + 1) * 128],
                                    Wt[:, s, nb * 512 : (nb + 1) * 512],
                                    start=(s == 0),
                                    stop=(s == KC - 1),
                                )
                    for h in range(2):
                        mb = 2 * p + h
                        for nb in range(NB):
                            nc.vector.tensor_copy(
                                rows[h][:, nb * 512 : (nb + 1) * 512],
                                accs[h * NB + nb][:],
                            )
                        nc.sync.dma_start(
                            out[mb * 128 : (mb + 1) * 128, :], rows[h][:]
                        )
                else:
                    # last pair: chain-major so copies + stores drain while
                    # later chains still run (kills the end-of-kernel tail)
                    for h in range(2):
                        mb = 2 * p + h
                        for nb in range(NB):
                            acc = accs[h * NB + nb]
                            for s in range(KC):
                                nc.tensor.matmul(
                                    acc[:],
                                    At[:, s, mb * 128 : (mb + 1) * 128],
                                    Wt[:, s, nb * 512 : (nb + 1) * 512],
                                    start=(s == 0),
                                    stop=(s == KC - 1),
                                )
                            nc.vector.tensor_copy(
                                rows[h][:, nb * 512 : (nb + 1) * 512], acc[:]
                            )
                            nc.sync.dma_start(
                                out[
                                    mb * 128 : (mb + 1) * 128,
                                    nb * 512 : (nb + 1) * 512,
                                ],
                                rows[h][:, nb * 512 : (nb + 1) * 512],
                            )
    nc.compile()
    _cached_nc = nc
    return nc


def shard_inputs(A, weight):
    A = np.asarray(A, dtype=np.float32)
    weight = np.asarray(weight, dtype=np.float32)
    bf = ml_dtypes.bfloat16
    # K-major concat over ranks: row s*K_LOC + k of W_t is weight[s, :, k].
    W_t = weight.transpose(0, 2, 1).astype(bf).reshape(K_TOT, N)
    in_maps = []
    for r in range(WORLD):
        A_t = (
            A[:, r * M_PER : (r + 1) * M_PER, :]
            .transpose(0, 2, 1)
            .astype(bf)
            .reshape(K_TOT, M_PER)
        )
        in_maps.append({"A": A_t, "W": W_t})
    return in_maps


def kernel(A, weight):
    nc = _build()
    in_maps = shard_inputs(A, weight)
    res = run_bass_kernel_spmd(nc, in_maps, core_ids=list(range(WORLD)))
    return np.stack(
        [res.results[r]["out"].astype(np.float32) for r in range(WORLD)], axis=0
    )
